# revision 7
# baseline (speedup 1.0000x reference)
"""Causal self-attention (B=2, S=2048, H=1024, NH=16) on 8 TRN2 NeuronCores.

Sharding: core c handles batch b = c//4 and heads [4*(c%4), 4*(c%4)+4).
Tensor-parallel c_attn (column split) + c_proj (row split) with a 4-way
ReduceScatter per batch group after c_proj; host concatenates the slices.

Per-core dataflow (all matmuls on the PE array):
  1. x[b] -> x^T via PE transposes (contraction over H needs H on partitions).
  2. QKV: Q^T,K^T [dk, S] and V [S, dk] for its 4 heads (Wq pre-scaled 1/8).
  3. Attention per head in "transposed space": S^T[k,q] = K-block @ Q^T,
     exp on ACT, causal corner mask on GPSIMD, then out^T = [V|1].T @ A_un^T.
     The appended ones-column makes the softmax denominators fall out of the
     same matmul (PSUM row 64); normalization is a DMA broadcast + one
     multiply on the tiny O^T, not a pass over the scores.
  4. Partial c_proj for the full batch, ReduceScatter(add) over the 4-core
     batch group; each rank keeps its 512-row slice.
"""

import sys

sys.path.insert(0, "/opt/trn_rl_repo")

import numpy as np

import concourse.bass as bass
import concourse.mybir as mybir
import concourse.tile as tile
from concourse import bacc
from concourse.bass_utils import run_bass_kernel_spmd
from concourse.masks import make_identity

B, S, H, NH, DK = 2, 2048, 1024, 16, 64
NCORES = 8
HPC = 4            # heads per core
CW = HPC * DK      # 256 qkv columns per core
SLICE = S // 4     # 512 output rows per core
GROUPS = [[0, 1, 2, 3], [4, 5, 6, 7]]

F32 = mybir.dt.float32
DT_MM = mybir.dt.float32  # matmul operand dtype: float32 | float32r


def _bcast_ap(src_ap, parts):
    """Partition-broadcast view: repeat src_ap's single row across `parts`."""
    ap = [list(p) for p in src_ap.ap]
    if len(ap) > 1 and ap[0][1] == 1:
        ap = ap[1:]  # drop singleton partition dim
    return bass.AP(
        tensor=src_ap.tensor,
        offset=src_ap.offset,
        ap=[[0, parts]] + ap,
    )


def build_nc(dt_mm=DT_MM):
    nc = bacc.Bacc(None, target_bir_lowering=False, debug=False, num_devices=NCORES)

    xb = nc.declare_dram_parameter("xb", [S, H], dt_mm, isOutput=False)
    wq = nc.declare_dram_parameter("wq", [H, CW], dt_mm, isOutput=False)
    wk = nc.declare_dram_parameter("wk", [H, CW], dt_mm, isOutput=False)
    wv = nc.declare_dram_parameter("wv", [H, CW], dt_mm, isOutput=False)
    wp = nc.declare_dram_parameter("wp", [CW, H], dt_mm, isOutput=False)
    bq = nc.declare_dram_parameter("bq", [CW], F32, isOutput=False)
    bk = nc.declare_dram_parameter("bk", [CW], F32, isOutput=False)
    bv = nc.declare_dram_parameter("bv", [CW], F32, isOutput=False)
    out = nc.declare_dram_parameter("out", [SLICE, H], F32, isOutput=True)

    KT = H // 128   # 8 contraction tiles over H
    ST = S // 128   # 16 seq tiles
    NQ = S // 512   # 4 query chunks of 512

    with tile.TileContext(nc) as tc:
        with (
            tc.tile_pool(name="dram", bufs=1, space="DRAM") as dram,
            tc.tile_pool(name="psum", bufs=1, space="PSUM") as psum,
            tc.tile_pool(name="persist", bufs=1) as pw,
        ):
            yp = dram.tile([S, H], F32)
            rs_out = dram.tile([SLICE, H], F32)
            sums_d = dram.tile([HPC, S], F32)

            ident = pw.tile([128, 128], dt_mm)
            make_identity(nc, ident)

            # weights: [128, k-tile, cols]
            wq_sb = pw.tile([128, KT, CW], dt_mm)
            wk_sb = pw.tile([128, KT, CW], dt_mm)
            wv_sb = pw.tile([128, KT, CW], dt_mm)
            wp_sb = pw.tile([128, 2, H], dt_mm)
            nc.sync.dma_start(out=wq_sb, in_=wq.ap().rearrange("(k p) c -> p k c", p=128))
            nc.sync.dma_start(out=wk_sb, in_=wk.ap().rearrange("(k p) c -> p k c", p=128))
            nc.sync.dma_start(out=wv_sb, in_=wv.ap().rearrange("(k p) c -> p k c", p=128))
            nc.sync.dma_start(out=wp_sb, in_=wp.ap().rearrange("(k p) c -> p k c", p=128))

            # biases
            bq_sb = pw.tile([128, 2], F32)
            bk_sb = pw.tile([128, 2], F32)
            nc.gpsimd.dma_start(out=bq_sb, in_=bq.ap().rearrange("(h p) -> p h", p=128))
            nc.gpsimd.dma_start(out=bk_sb, in_=bk.ap().rearrange("(h p) -> p h", p=128))
            bv_bc = pw.tile([128, CW], F32)
            nc.gpsimd.dma_start(out=bv_bc, in_=_bcast_ap(bv.ap(), 128))

            QTt = pw.tile([128, 2, S], dt_mm)   # q-col (128) x [half, s]
            KTt = pw.tile([128, 2, S], dt_mm)
            V4 = pw.tile([128, ST, HPC, DK + 1], dt_mm)  # [s-part, s-tile, head, dk|1]

            # ---- phase 0+1: x^T and QKV ----
            with tc.tile_pool(name="px", bufs=1) as px:
                xT = px.tile([128, KT, S], dt_mm)  # h-part x [h-tile, s]
                for si in range(ST):
                    xs = px.tile([128, H], dt_mm, tag="xs", bufs=3)
                    nc.sync.dma_start(out=xs, in_=xb[si * 128:(si + 1) * 128, :])
                    for k in range(KT):
                        pt = psum.tile([128, 128], dt_mm, tag="tp", bufs=2)
                        nc.tensor.transpose(pt, xs[:, k * 128:(k + 1) * 128], ident)
                        nc.vector.tensor_copy(xT[:, k, si * 128:(si + 1) * 128], pt)

                # Q^T, K^T: [128 cols, 512 q] tiles, accumulate over H
                for (wt, dst, bias) in ((wq_sb, QTt, bq_sb), (wk_sb, KTt, bk_sb)):
                    for j in range(NQ):
                        for half in range(2):
                            pq = psum.tile([128, 512], F32, tag="qkv", bufs=2)
                            for k in range(KT):
                                nc.tensor.matmul(
                                    pq,
                                    wt[:, k, half * 128:(half + 1) * 128],
                                    xT[:, k, j * 512:(j + 1) * 512],
                                    start=(k == 0),
                                    stop=(k == KT - 1),
                                )
                            nc.scalar.activation(
                                dst[:, half, j * 512:(j + 1) * 512], pq,
                                mybir.ActivationFunctionType.Identity,
                                bias=bias[:, half:half + 1],
                            )

                # V: [s-tile 128, 256] + bias, packed as [V|1] per head
                for si in range(ST):
                    pv = psum.tile([128, CW], F32, tag="qkv", bufs=2)
                    for k in range(KT):
                        nc.tensor.matmul(
                            pv, xT[:, k, si * 128:(si + 1) * 128], wv_sb[:, k, :],
                            start=(k == 0), stop=(k == KT - 1),
                        )
                    pv_h = pv[:, :].rearrange("p (h d) -> p h d", h=HPC)
                    bv_h = bv_bc[:, :].rearrange("p (h d) -> p h d", h=HPC)
                    nc.vector.tensor_add(V4[:, si, :, 0:DK], pv_h, bv_h)
                    nc.gpsimd.memset(V4[:, si, :, DK:DK + 1], 1.0)

            # ---- phase 2: attention per head ----
            with tc.tile_pool(name="patt", bufs=1) as patt:
                OTP = [patt.tile([128, S], F32, tag=f"otp{p}", name=f"otp{p}") for p in range(2)]
                for h in range(HPC):
                    pair, odd = h // 2, h % 2
                    pb = 64 * odd  # partition base of this head's Q^T/K^T rows
                    qt = QTt[pb:pb + DK, h // 2, :]
                    kt = KTt[pb:pb + DK, h // 2, :]
                    comb = patt.tile([65, S], F32, tag="comb", bufs=2)
                    for j in range(NQ):
                        pav = psum.tile([65, 512], F32, tag="av", bufs=2)
                        nblk = 4 * j + 4
                        for ki in range(nblk):
                            off = max(0, 128 * ki - 512 * j)
                            npp = 512 - off
                            ps = psum.tile([128, 512], F32, tag="sT", bufs=2)
                            nc.tensor.matmul(
                                ps[:, :npp],
                                kt[:, ki * 128:(ki + 1) * 128],
                                qt[:, j * 512 + off:(j + 1) * 512],
                                start=True, stop=True,
                            )
                            A = patt.tile([128, 512], dt_mm, tag="A", bufs=3)
                            nc.scalar.activation(
                                A[:, :npp], ps[:, :npp],
                                mybir.ActivationFunctionType.Exp)
                            if ki >= 4 * j:
                                # corner: keep q_local - k_local >= 0 else 0
                                nc.gpsimd.affine_select(
                                    out=A[:, :128], in_=A[:, :128],
                                    compare_op=mybir.AluOpType.is_ge,
                                    fill=0.0, base=0,
                                    pattern=[[1, 128]], channel_multiplier=-1,
                                )
                            nc.tensor.matmul(
                                pav[:, off:],
                                V4[:, ki, h, :],
                                A[:, :npp],
                                start=(ki == 0), stop=(ki == nblk - 1),
                            )
                        nc.vector.tensor_copy(
                            comb[:, j * 512:(j + 1) * 512], pav)
                    # softmax denominators: reciprocal, broadcast via DRAM
                    nc.vector.reciprocal(comb[64:65, :], comb[64:65, :])
                    nc.sync.dma_start(out=sums_d[h:h + 1, :], in_=comb[64:65, :])
                    rbc = patt.tile([64, S], F32, tag="rbc", bufs=2)
                    nc.gpsimd.dma_start(
                        out=rbc, in_=_bcast_ap(sums_d[h:h + 1, :], 64))
                    nc.vector.tensor_mul(comb[0:64, :], comb[0:64, :], rbc)
                    # pack the head pair: partitions [64*odd, 64*odd+64)
                    nc.sync.dma_start(
                        out=OTP[pair][pb:pb + 64, :], in_=comb[0:64, :])

                # ---- phase 3: partial c_proj for the full batch ----
                with tc.tile_pool(name="pproj", bufs=1) as pproj:
                    otmm = OTP
                    if dt_mm != F32:
                        otmm = []
                        for pair in range(2):
                            t = pproj.tile([128, S], dt_mm, tag=f"otmm{pair}", name=f"otmm{pair}")
                            nc.vector.tensor_copy(t, OTP[pair])
                            otmm.append(t)
                    for si in range(ST):
                        yt = pproj.tile([128, H], F32, tag="y", bufs=3)
                        for n in range(2):
                            py = psum.tile([128, 512], F32, tag="qkv", bufs=2)
                            for pair in range(2):
                                nc.tensor.matmul(
                                    py,
                                    otmm[pair][:, si * 128:(si + 1) * 128],
                                    wp_sb[:, pair, n * 512:(n + 1) * 512],
                                    start=(pair == 0), stop=(pair == 1),
                                )
                            nc.vector.tensor_copy(yt[:, n * 512:(n + 1) * 512], py)
                        nc.sync.dma_start(
                            out=yp[si * 128:(si + 1) * 128, :], in_=yt)

            nc.gpsimd.collective_compute(
                "ReduceScatter",
                mybir.AluOpType.add,
                replica_groups=GROUPS,
                ins=[yp[:, :].opt()],
                outs=[rs_out[:, :].opt()],
            )
            nc.gpsimd.dma_start(out=out[:, :], in_=rs_out[:, :])

    nc.compile()
    return nc


_NC = None


def kernel(x, w_attn, b_attn, w_proj, b_proj):
    global _NC
    if _NC is None:
        _NC = build_nc()

    x = np.ascontiguousarray(np.asarray(x, dtype=np.float32))
    w_attn = np.asarray(w_attn, dtype=np.float32)
    b_attn = np.asarray(b_attn, dtype=np.float32)
    w_proj = np.asarray(w_proj, dtype=np.float32)
    b_proj = np.asarray(b_proj, dtype=np.float32)

    in_maps = []
    for c in range(NCORES):
        b, g = divmod(c, 4)
        h0 = g * HPC
        cs = slice(h0 * DK, h0 * DK + CW)
        in_maps.append({
            "xb": x[b],
            # fold the 1/sqrt(DK)=2^-3 score scale into Wq/bq (exact in fp32)
            "wq": np.ascontiguousarray(w_attn[:, cs]) * np.float32(0.125),
            "wk": np.ascontiguousarray(w_attn[:, H:][:, cs]),
            "wv": np.ascontiguousarray(w_attn[:, 2 * H:][:, cs]),
            "wp": np.ascontiguousarray(w_proj[cs.start:cs.stop, :]),
            "bq": np.ascontiguousarray(b_attn[cs]) * np.float32(0.125),
            "bk": np.ascontiguousarray(b_attn[H:][cs]),
            "bv": np.ascontiguousarray(b_attn[2 * H:][cs]),
        })

    res = run_bass_kernel_spmd(_NC, in_maps, core_ids=list(range(NCORES)))

    outp = np.empty((B, S, H), dtype=np.float32)
    for c in range(NCORES):
        b, g = divmod(c, 4)
        outp[b, g * SLICE:(g + 1) * SLICE, :] = res.results[c]["out"]
    outp += b_proj  # row-broadcast add, exact
    return outp


# revision 10
# speedup vs baseline: 1.6051x; 1.6051x over previous
"""Causal self-attention (B=2, S=2048, H=1024, NH=16) on 8 TRN2 NeuronCores.

Sharding: core c handles batch b = c//4 and heads [4*(c%4), 4*(c%4)+4).
Tensor-parallel c_attn (column split) + c_proj (row split) with a 4-way
ReduceScatter per batch group after c_proj; host concatenates the slices.

Per-core dataflow (all matmuls on the PE array):
  1. x[b] -> x^T via PE transposes (contraction over H needs H on partitions).
  2. QKV: Q^T,K^T [dk, S] and V [S, dk] for its 4 heads (Wq pre-scaled 1/8).
  3. Attention per head in "transposed space": S^T[k,q] = K-block @ Q^T,
     exp on ACT, causal corner mask on GPSIMD, then out^T = [V|1].T @ A_un^T.
     The appended ones-column makes the softmax denominators fall out of the
     same matmul (PSUM row 64); normalization is a DMA broadcast + one
     multiply on the tiny O^T, not a pass over the scores.
  4. Partial c_proj for the full batch, ReduceScatter(add) over the 4-core
     batch group; each rank keeps its 512-row slice.
"""

import sys

sys.path.insert(0, "/opt/trn_rl_repo")

import numpy as np

import concourse.bass as bass
import concourse.mybir as mybir
import concourse.tile as tile
from concourse import bacc
from concourse.bass_utils import run_bass_kernel_spmd
from concourse.masks import make_identity

B, S, H, NH, DK = 2, 2048, 1024, 16, 64
NCORES = 8
HPC = 4            # heads per core
CW = HPC * DK      # 256 qkv columns per core
SLICE = S // 4     # 512 output rows per core
GROUPS = [[0, 1, 2, 3], [4, 5, 6, 7]]

F32 = mybir.dt.float32
DT_MM = mybir.dt.float32r  # matmul operand dtype: float32 | float32r


def _bcast_ap(src_ap, parts):
    """Partition-broadcast view: repeat src_ap's single row across `parts`."""
    ap = [list(p) for p in src_ap.ap]
    if len(ap) > 1 and ap[0][1] == 1:
        ap = ap[1:]  # drop singleton partition dim
    return bass.AP(
        tensor=src_ap.tensor,
        offset=src_ap.offset,
        ap=[[0, parts]] + ap,
    )


def build_nc(dt_mm=DT_MM):
    nc = bacc.Bacc(None, target_bir_lowering=False, debug=False, num_devices=NCORES)

    xb = nc.declare_dram_parameter("xb", [S, H], dt_mm, isOutput=False)
    wq = nc.declare_dram_parameter("wq", [H, CW], dt_mm, isOutput=False)
    wk = nc.declare_dram_parameter("wk", [H, CW], dt_mm, isOutput=False)
    wv = nc.declare_dram_parameter("wv", [H, CW], dt_mm, isOutput=False)
    wp = nc.declare_dram_parameter("wp", [CW, H], dt_mm, isOutput=False)
    bq = nc.declare_dram_parameter("bq", [CW], F32, isOutput=False)
    bk = nc.declare_dram_parameter("bk", [CW], F32, isOutput=False)
    bv = nc.declare_dram_parameter("bv", [CW], F32, isOutput=False)
    out = nc.declare_dram_parameter("out", [SLICE, H], F32, isOutput=True)

    KT = H // 128   # 8 contraction tiles over H
    ST = S // 128   # 16 seq tiles
    NQ = S // 512   # 4 query chunks of 512

    with tile.TileContext(nc) as tc:
        with (
            tc.tile_pool(name="dram", bufs=1, space="DRAM") as dram,
            tc.tile_pool(name="psum", bufs=1, space="PSUM") as psum,
            tc.tile_pool(name="persist", bufs=1) as pw,
        ):
            yp = dram.tile([S, H], F32)
            rs_out = dram.tile([SLICE, H], F32)
            sums_d = dram.tile([HPC, S], F32)

            ident = pw.tile([128, 128], dt_mm)
            if dt_mm != F32:
                ident_f32 = pw.tile([128, 128], F32)
                make_identity(nc, ident_f32)
                nc.vector.tensor_copy(ident, ident_f32)
            else:
                make_identity(nc, ident)
            ones4 = pw.tile([128, HPC, 1], F32)
            nc.gpsimd.memset(ones4, 1.0)

            # weights: [128, k-tile, cols]
            wq_sb = pw.tile([128, KT, CW], dt_mm)
            wk_sb = pw.tile([128, KT, CW], dt_mm)
            wv_sb = pw.tile([128, KT, CW], dt_mm)
            wp_sb = pw.tile([128, 2, H], dt_mm)
            nc.sync.dma_start(out=wq_sb, in_=wq.ap().rearrange("(k p) c -> p k c", p=128))
            nc.sync.dma_start(out=wk_sb, in_=wk.ap().rearrange("(k p) c -> p k c", p=128))
            nc.sync.dma_start(out=wv_sb, in_=wv.ap().rearrange("(k p) c -> p k c", p=128))
            nc.sync.dma_start(out=wp_sb, in_=wp.ap().rearrange("(k p) c -> p k c", p=128))

            # biases
            bq_sb = pw.tile([128, 2], F32)
            bk_sb = pw.tile([128, 2], F32)
            nc.gpsimd.dma_start(out=bq_sb, in_=bq.ap().rearrange("(h p) -> p h", p=128))
            nc.gpsimd.dma_start(out=bk_sb, in_=bk.ap().rearrange("(h p) -> p h", p=128))
            bv_bc = pw.tile([128, CW], F32)
            nc.gpsimd.dma_start(out=bv_bc, in_=_bcast_ap(bv.ap(), 128))

            QTt = pw.tile([128, 2, S], dt_mm)   # q-col (128) x [half, s]
            KTt = pw.tile([128, 2, S], dt_mm)
            V4 = pw.tile([128, ST, HPC, DK + 1], dt_mm)  # [s-part, s-tile, head, dk|1]

            # ---- phase 0+1: x^T and QKV ----
            with tc.tile_pool(name="px", bufs=1) as px:
                xT = px.tile([128, KT, S], dt_mm)  # h-part x [h-tile, s]
                for si in range(ST):
                    xs = px.tile([128, H], dt_mm, tag="xs", bufs=3)
                    nc.sync.dma_start(out=xs, in_=xb[si * 128:(si + 1) * 128, :])
                    for k in range(KT):
                        pt = psum.tile([128, 128], dt_mm, tag="tp", bufs=2)
                        nc.tensor.transpose(pt, xs[:, k * 128:(k + 1) * 128], ident)
                        nc.vector.tensor_copy(xT[:, k, si * 128:(si + 1) * 128], pt)

                # Q^T, K^T: [128 cols, 512 q] tiles, accumulate over H
                for (wt, dst, bias) in ((wq_sb, QTt, bq_sb), (wk_sb, KTt, bk_sb)):
                    for j in range(NQ):
                        for half in range(2):
                            pq = psum.tile([128, 512], F32, tag="qkv", bufs=2)
                            for k in range(KT):
                                nc.tensor.matmul(
                                    pq,
                                    wt[:, k, half * 128:(half + 1) * 128],
                                    xT[:, k, j * 512:(j + 1) * 512],
                                    start=(k == 0),
                                    stop=(k == KT - 1),
                                )
                            nc.scalar.activation(
                                dst[:, half, j * 512:(j + 1) * 512], pq,
                                mybir.ActivationFunctionType.Identity,
                                bias=bias[:, half:half + 1],
                            )

                # V: [s-tile 128, 256] + bias, packed as [V|1] per head
                for si in range(ST):
                    pv = psum.tile([128, CW], F32, tag="qkv", bufs=2)
                    for k in range(KT):
                        nc.tensor.matmul(
                            pv, xT[:, k, si * 128:(si + 1) * 128], wv_sb[:, k, :],
                            start=(k == 0), stop=(k == KT - 1),
                        )
                    pv_h = pv[:, :].rearrange("p (h d) -> p h d", h=HPC)
                    bv_h = bv_bc[:, :].rearrange("p (h d) -> p h d", h=HPC)
                    nc.vector.tensor_add(V4[:, si, :, 0:DK], pv_h, bv_h)
                    nc.vector.tensor_copy(V4[:, si, :, DK:DK + 1], ones4)

            # ---- phase 2: attention per head ----
            with tc.tile_pool(name="patt", bufs=1) as patt:
                OTP = [patt.tile([128, S], F32, tag=f"otp{p}", name=f"otp{p}") for p in range(2)]
                for h in range(HPC):
                    pair, odd = h // 2, h % 2
                    pb = 64 * odd  # partition base of this head's Q^T/K^T rows
                    qt = QTt[pb:pb + DK, h // 2, :]
                    kt = KTt[pb:pb + DK, h // 2, :]
                    comb = patt.tile([65, S], F32, tag="comb", bufs=2)
                    for j in range(NQ):
                        pav = psum.tile([65, 512], F32, tag="av", bufs=2)
                        nblk = 4 * j + 4
                        for ki in range(nblk):
                            off = max(0, 128 * ki - 512 * j)
                            npp = 512 - off
                            ps = psum.tile([128, 512], F32, tag="sT", bufs=2)
                            nc.tensor.matmul(
                                ps[:, :npp],
                                kt[:, ki * 128:(ki + 1) * 128],
                                qt[:, j * 512 + off:(j + 1) * 512],
                                start=True, stop=True,
                            )
                            A = patt.tile([128, 512], dt_mm, tag="A", bufs=3)
                            nc.scalar.activation(
                                A[:, :npp], ps[:, :npp],
                                mybir.ActivationFunctionType.Exp)
                            if ki >= 4 * j:
                                # corner: keep q_local - k_local >= 0 else 0
                                nc.gpsimd.affine_select(
                                    out=A[:, :128], in_=A[:, :128],
                                    compare_op=mybir.AluOpType.is_ge,
                                    fill=0.0, base=0,
                                    pattern=[[1, 128]], channel_multiplier=-1,
                                )
                            nc.tensor.matmul(
                                pav[:, off:],
                                V4[:, ki, h, :],
                                A[:, :npp],
                                start=(ki == 0), stop=(ki == nblk - 1),
                            )
                        nc.vector.tensor_copy(
                            comb[:, j * 512:(j + 1) * 512], pav)
                    # softmax denominators: reciprocal, broadcast via DRAM
                    nc.vector.reciprocal(comb[64:65, :], comb[64:65, :])
                    nc.sync.dma_start(out=sums_d[h:h + 1, :], in_=comb[64:65, :])
                    rbc = patt.tile([64, S], F32, tag="rbc", bufs=2)
                    nc.gpsimd.dma_start(
                        out=rbc, in_=_bcast_ap(sums_d[h:h + 1, :], 64))
                    nc.vector.tensor_mul(comb[0:64, :], comb[0:64, :], rbc)
                    # pack the head pair: partitions [64*odd, 64*odd+64)
                    nc.sync.dma_start(
                        out=OTP[pair][pb:pb + 64, :], in_=comb[0:64, :])

                # ---- phase 3: partial c_proj for the full batch ----
                with tc.tile_pool(name="pproj", bufs=1) as pproj:
                    otmm = OTP
                    if dt_mm != F32:
                        otmm = []
                        for pair in range(2):
                            t = pproj.tile([128, S], dt_mm, tag=f"otmm{pair}", name=f"otmm{pair}")
                            nc.vector.tensor_copy(t, OTP[pair])
                            otmm.append(t)
                    for si in range(ST):
                        yt = pproj.tile([128, H], F32, tag="y", bufs=3)
                        for n in range(2):
                            py = psum.tile([128, 512], F32, tag="qkv", bufs=2)
                            for pair in range(2):
                                nc.tensor.matmul(
                                    py,
                                    otmm[pair][:, si * 128:(si + 1) * 128],
                                    wp_sb[:, pair, n * 512:(n + 1) * 512],
                                    start=(pair == 0), stop=(pair == 1),
                                )
                            nc.vector.tensor_copy(yt[:, n * 512:(n + 1) * 512], py)
                        nc.sync.dma_start(
                            out=yp[si * 128:(si + 1) * 128, :], in_=yt)

            nc.gpsimd.collective_compute(
                "ReduceScatter",
                mybir.AluOpType.add,
                replica_groups=GROUPS,
                ins=[yp[:, :].opt()],
                outs=[rs_out[:, :].opt()],
            )
            nc.gpsimd.dma_start(out=out[:, :], in_=rs_out[:, :])

    nc.compile()
    return nc


_NC = None


def kernel(x, w_attn, b_attn, w_proj, b_proj):
    global _NC
    if _NC is None:
        _NC = build_nc()

    x = np.ascontiguousarray(np.asarray(x, dtype=np.float32))
    w_attn = np.asarray(w_attn, dtype=np.float32)
    b_attn = np.asarray(b_attn, dtype=np.float32)
    w_proj = np.asarray(w_proj, dtype=np.float32)
    b_proj = np.asarray(b_proj, dtype=np.float32)

    in_maps = []
    for c in range(NCORES):
        b, g = divmod(c, 4)
        h0 = g * HPC
        cs = slice(h0 * DK, h0 * DK + CW)
        in_maps.append({
            "xb": x[b],
            # fold the 1/sqrt(DK)=2^-3 score scale into Wq/bq (exact in fp32)
            "wq": np.ascontiguousarray(w_attn[:, cs]) * np.float32(0.125),
            "wk": np.ascontiguousarray(w_attn[:, H:][:, cs]),
            "wv": np.ascontiguousarray(w_attn[:, 2 * H:][:, cs]),
            "wp": np.ascontiguousarray(w_proj[cs.start:cs.stop, :]),
            "bq": np.ascontiguousarray(b_attn[cs]) * np.float32(0.125),
            "bk": np.ascontiguousarray(b_attn[H:][cs]),
            "bv": np.ascontiguousarray(b_attn[2 * H:][cs]),
        })

    res = run_bass_kernel_spmd(_NC, in_maps, core_ids=list(range(NCORES)))

    outp = np.empty((B, S, H), dtype=np.float32)
    for c in range(NCORES):
        b, g = divmod(c, 4)
        outp[b, g * SLICE:(g + 1) * SLICE, :] = res.results[c]["out"]
    outp += b_proj  # row-broadcast add, exact
    return outp


# revision 14
# speedup vs baseline: 1.6394x; 1.0213x over previous
"""Causal self-attention (B=2, S=2048, H=1024, NH=16) on 8 TRN2 NeuronCores.

Sharding: core c handles batch b = c//4 and heads [4*(c%4), 4*(c%4)+4).
Tensor-parallel c_attn (column split) + c_proj (row split) with a 4-way
ReduceScatter per batch group after c_proj; host concatenates the slices.

Per-core dataflow (all matmuls on the PE array):
  1. x[b] -> x^T via PE transposes (contraction over H needs H on partitions).
  2. QKV: Q^T,K^T [dk, S] and V [S, dk] for its 4 heads (Wq pre-scaled 1/8).
  3. Attention per head in "transposed space": S^T[k,q] = K-block @ Q^T,
     exp on ACT, causal corner mask on GPSIMD, then out^T = [V|1].T @ A_un^T.
     The appended ones-column makes the softmax denominators fall out of the
     same matmul (PSUM row 64); normalization is a DMA broadcast + one
     multiply on the tiny O^T, not a pass over the scores.
  4. Partial c_proj, ReduceScatter(add) over the 4-core batch group.

Everything after x^T is pipelined over 512-wide query chunks j: QKV(j) ->
attention(j) -> c_proj(j) -> ReduceScatter(j), so the collective overlaps
compute and the PE stream stays dense.
"""

import sys

sys.path.insert(0, "/opt/trn_rl_repo")

import numpy as np

import concourse.bass as bass
import concourse.mybir as mybir
import concourse.tile as tile
from concourse import bacc
from concourse.bass_utils import run_bass_kernel_spmd
from concourse.masks import make_identity

B, S, H, NH, DK = 2, 2048, 1024, 16, 64
NCORES = 8
HPC = 4            # heads per core
CW = HPC * DK      # 256 qkv columns per core
SLICE = S // 4     # 512 output rows per core
GROUPS = [[0, 1, 2, 3], [4, 5, 6, 7]]

F32 = mybir.dt.float32
DT_MM = mybir.dt.float32r  # matmul operand dtype: float32 | float32r


def _bcast_ap(src_ap, parts):
    """Partition-broadcast view: repeat src_ap's single row across `parts`."""
    ap = [list(p) for p in src_ap.ap]
    if len(ap) > 1 and ap[0][1] == 1:
        ap = ap[1:]  # drop singleton partition dim
    return bass.AP(
        tensor=src_ap.tensor,
        offset=src_ap.offset,
        ap=[[0, parts]] + ap,
    )


def build_nc(dt_mm=DT_MM):
    nc = bacc.Bacc(None, target_bir_lowering=False, debug=False, num_devices=NCORES)

    xb = nc.declare_dram_parameter("xb", [S, H], dt_mm, isOutput=False)
    wq = nc.declare_dram_parameter("wq", [H, CW], dt_mm, isOutput=False)
    wk = nc.declare_dram_parameter("wk", [H, CW], dt_mm, isOutput=False)
    wv = nc.declare_dram_parameter("wv", [H, CW], dt_mm, isOutput=False)
    wp = nc.declare_dram_parameter("wp", [CW, H], dt_mm, isOutput=False)
    bq = nc.declare_dram_parameter("bq", [CW], F32, isOutput=False)
    bk = nc.declare_dram_parameter("bk", [CW], F32, isOutput=False)
    bv = nc.declare_dram_parameter("bv", [CW], F32, isOutput=False)
    out = nc.declare_dram_parameter("out", [SLICE, H], F32, isOutput=True)

    KT = H // 128   # 8 contraction tiles over H
    ST = S // 128   # 16 seq tiles
    NQ = S // 512   # 4 query chunks of 512

    with tile.TileContext(nc) as tc:
        with (
            tc.tile_pool(name="dram", bufs=1, space="DRAM") as dram,
            tc.tile_pool(name="psum", bufs=1, space="PSUM") as psum,
            tc.tile_pool(name="persist", bufs=1) as pw,
        ):
            ident = pw.tile([128, 128], dt_mm)
            if dt_mm != F32:
                ident_f32 = pw.tile([128, 128], F32)
                make_identity(nc, ident_f32)
                nc.vector.tensor_copy(ident, ident_f32)
            else:
                make_identity(nc, ident)
            ones4 = pw.tile([128, HPC, 1], F32)
            nc.gpsimd.memset(ones4, 1.0)

            # weights: [128, k-tile, cols]
            wq_sb = pw.tile([128, KT, CW], dt_mm)
            wk_sb = pw.tile([128, KT, CW], dt_mm)
            wv_sb = pw.tile([128, KT, CW], dt_mm)
            wp_sb = pw.tile([128, 2, H], dt_mm)
            nc.sync.dma_start(out=wq_sb, in_=wq.ap().rearrange("(k p) c -> p k c", p=128))
            nc.sync.dma_start(out=wk_sb, in_=wk.ap().rearrange("(k p) c -> p k c", p=128))
            nc.sync.dma_start(out=wv_sb, in_=wv.ap().rearrange("(k p) c -> p k c", p=128))
            nc.sync.dma_start(out=wp_sb, in_=wp.ap().rearrange("(k p) c -> p k c", p=128))

            # biases
            bq_sb = pw.tile([128, 2], F32)
            bk_sb = pw.tile([128, 2], F32)
            nc.gpsimd.dma_start(out=bq_sb, in_=bq.ap().rearrange("(h p) -> p h", p=128))
            nc.gpsimd.dma_start(out=bk_sb, in_=bk.ap().rearrange("(h p) -> p h", p=128))
            bv_bc = pw.tile([128, CW], F32)
            nc.gpsimd.dma_start(out=bv_bc, in_=_bcast_ap(bv.ap(), 128))

            QTt = pw.tile([128, 2, S], dt_mm)   # q-col (128) x [half, s]
            KTt = pw.tile([128, 2, S], dt_mm)
            V4 = pw.tile([128, ST, HPC, DK + 1], dt_mm)  # [s-part, s-tile, head, dk|1]
            xT = pw.tile([128, KT, S], dt_mm)   # h-part x [h-tile, s]
            OTP = [pw.tile([128, S], dt_mm, name=f"otp{p}") for p in range(2)]

            # ---- phase 0: x^T ----
            for si in range(ST):
                xs = pw.tile([128, H], dt_mm, tag="xs", bufs=2)
                nc.sync.dma_start(out=xs, in_=xb[si * 128:(si + 1) * 128, :])
                for k in range(KT):
                    pt = psum.tile([128, 128], dt_mm, tag="tp", bufs=2)
                    nc.tensor.transpose(pt, xs[:, k * 128:(k + 1) * 128], ident)
                    nc.vector.tensor_copy(xT[:, k, si * 128:(si + 1) * 128], pt)

            # ---- phases 1-3, pipelined over query chunks j ----
            for j in range(NQ):
                js = slice(j * 512, (j + 1) * 512)

                # QKV for chunk j
                for (wt, dst, bias) in ((wq_sb, QTt, bq_sb), (wk_sb, KTt, bk_sb)):
                    for half in range(2):
                        pq = psum.tile([128, 512], F32, tag="qkv", bufs=2)
                        for k in range(KT):
                            nc.tensor.matmul(
                                pq,
                                wt[:, k, half * 128:(half + 1) * 128],
                                xT[:, k, js],
                                start=(k == 0),
                                stop=(k == KT - 1),
                            )
                        nc.scalar.activation(
                            dst[:, half, js], pq,
                            mybir.ActivationFunctionType.Identity,
                            bias=bias[:, half:half + 1],
                        )
                for si in range(4 * j, 4 * j + 4):
                    pv = psum.tile([128, CW], F32, tag="qkv", bufs=2)
                    for k in range(KT):
                        nc.tensor.matmul(
                            pv, xT[:, k, si * 128:(si + 1) * 128], wv_sb[:, k, :],
                            start=(k == 0), stop=(k == KT - 1),
                        )
                    pv_h = pv[:, :].rearrange("p (h d) -> p h d", h=HPC)
                    bv_h = bv_bc[:, :].rearrange("p (h d) -> p h d", h=HPC)
                    nc.vector.tensor_add(V4[:, si, :, 0:DK], pv_h, bv_h)
                    nc.vector.tensor_copy(V4[:, si, :, DK:DK + 1], ones4)

                # attention chunk j, all 4 heads
                combs = []
                for h in range(HPC):
                    pb = 64 * (h % 2)
                    qt = QTt[pb:pb + DK, h // 2, :]
                    kt = KTt[pb:pb + DK, h // 2, :]
                    comb = pw.tile([65, 512], dt_mm, tag="comb", bufs=5,
                                   name=f"comb{j}_{h}")
                    combs.append(comb)
                    pav = psum.tile([65, 512], F32, tag="av", bufs=2)
                    nblk = 4 * j + 4
                    for ki in range(nblk):
                        off = max(0, 128 * ki - 512 * j)
                        npp = 512 - off
                        ps = psum.tile([128, 512], F32, tag="sT", bufs=2)
                        nc.tensor.matmul(
                            ps[:, :npp],
                            kt[:, ki * 128:(ki + 1) * 128],
                            qt[:, j * 512 + off:(j + 1) * 512],
                            start=True, stop=True,
                        )
                        A = pw.tile([128, 512], dt_mm, tag="A", bufs=3)
                        nc.scalar.activation(
                            A[:, :npp], ps[:, :npp],
                            mybir.ActivationFunctionType.Exp)
                        if ki >= 4 * j:
                            # corner: keep q_local - k_local >= 0 else 0
                            nc.gpsimd.affine_select(
                                out=A[:, :128], in_=A[:, :128],
                                compare_op=mybir.AluOpType.is_ge,
                                fill=0.0, base=0,
                                pattern=[[1, 128]], channel_multiplier=-1,
                            )
                        nc.tensor.matmul(
                            pav[:, off:],
                            V4[:, ki, h, :],
                            A[:, :npp],
                            start=(ki == 0), stop=(ki == nblk - 1),
                        )
                    nc.vector.tensor_copy(comb, pav)

                # normalization for chunk j: batch the 4 heads' reciprocals
                sums_j = dram.tile([HPC * 512], dt_mm, name=f"sums{j}")
                rsums_j = dram.tile([HPC * 512], dt_mm, name=f"rsums{j}")
                for h in range(HPC):
                    nc.sync.dma_start(
                        out=sums_j[h * 512:(h + 1) * 512], in_=combs[h][64:65, :])
                sre = pw.tile([128, HPC * 4], dt_mm, tag="sre", bufs=2)
                nc.sync.dma_start(
                    out=sre, in_=sums_j[:].rearrange("(p f) -> p f", p=128))
                with nc.allow_low_precision(reason="f32r recip of O(1e3) softmax sums"):
                    nc.vector.reciprocal(sre, sre)
                nc.sync.dma_start(
                    out=rsums_j[:].rearrange("(p f) -> p f", p=128), in_=sre)
                for h in range(HPC):
                    pair, pb = h // 2, 64 * (h % 2)
                    rbc = pw.tile([64, 512], dt_mm, tag="rbc", bufs=3)
                    nc.gpsimd.dma_start(
                        out=rbc, in_=_bcast_ap(rsums_j[h * 512:(h + 1) * 512], 64))
                    nc.vector.tensor_mul(combs[h][0:64, :], combs[h][0:64, :], rbc)
                    nc.sync.dma_start(
                        out=OTP[pair][pb:pb + 64, js], in_=combs[h][0:64, :])

                # c_proj chunk j + ReduceScatter
                yp_j = dram.tile([512, H], F32, name=f"yp{j}")
                rs_j = dram.tile([128, H], F32, name=f"rs{j}")
                for si in range(4 * j, 4 * j + 4):
                    yt = pw.tile([128, H], F32, tag="y", bufs=2)
                    for n in range(2):
                        py = psum.tile([128, 512], F32, tag="qkv", bufs=2)
                        for pair in range(2):
                            nc.tensor.matmul(
                                py,
                                OTP[pair][:, si * 128:(si + 1) * 128],
                                wp_sb[:, pair, n * 512:(n + 1) * 512],
                                start=(pair == 0), stop=(pair == 1),
                            )
                        nc.vector.tensor_copy(yt[:, n * 512:(n + 1) * 512], py)
                    nc.sync.dma_start(
                        out=yp_j[(si - 4 * j) * 128:(si - 4 * j + 1) * 128, :],
                        in_=yt)
                nc.gpsimd.collective_compute(
                    "ReduceScatter",
                    mybir.AluOpType.add,
                    replica_groups=GROUPS,
                    ins=[yp_j[:, :].opt()],
                    outs=[rs_j[:, :].opt()],
                )
                nc.gpsimd.dma_start(
                    out=out[j * 128:(j + 1) * 128, :], in_=rs_j[:, :])

    nc.compile()
    return nc


_NC = None


def kernel(x, w_attn, b_attn, w_proj, b_proj):
    global _NC
    if _NC is None:
        _NC = build_nc()

    x = np.ascontiguousarray(np.asarray(x, dtype=np.float32))
    w_attn = np.asarray(w_attn, dtype=np.float32)
    b_attn = np.asarray(b_attn, dtype=np.float32)
    w_proj = np.asarray(w_proj, dtype=np.float32)
    b_proj = np.asarray(b_proj, dtype=np.float32)

    in_maps = []
    for c in range(NCORES):
        b, g = divmod(c, 4)
        h0 = g * HPC
        cs = slice(h0 * DK, h0 * DK + CW)
        in_maps.append({
            "xb": x[b],
            # fold the 1/sqrt(DK)=2^-3 score scale into Wq/bq (exact in fp32)
            "wq": np.ascontiguousarray(w_attn[:, cs]) * np.float32(0.125),
            "wk": np.ascontiguousarray(w_attn[:, H:][:, cs]),
            "wv": np.ascontiguousarray(w_attn[:, 2 * H:][:, cs]),
            "wp": np.ascontiguousarray(w_proj[cs.start:cs.stop, :]),
            "bq": np.ascontiguousarray(b_attn[cs]) * np.float32(0.125),
            "bk": np.ascontiguousarray(b_attn[H:][cs]),
            "bv": np.ascontiguousarray(b_attn[2 * H:][cs]),
        })

    res = run_bass_kernel_spmd(_NC, in_maps, core_ids=list(range(NCORES)))

    outp = np.empty((B, S, H), dtype=np.float32)
    for c in range(NCORES):
        b, g = divmod(c, 4)
        # out rows (j, r): global s = 512*j + 128*g + r
        arr = res.results[c]["out"].reshape(4, 128, H)
        for j in range(4):
            outp[b, 512 * j + 128 * g:512 * j + 128 * g + 128, :] = arr[j]
    outp += b_proj  # row-broadcast add, exact
    return outp


# revision 16
# speedup vs baseline: 1.8705x; 1.1410x over previous
"""Causal self-attention (B=2, S=2048, H=1024, NH=16) on 8 TRN2 NeuronCores.

Sharding: core c handles batch b = c//4 and heads [4*(c%4), 4*(c%4)+4).
Tensor-parallel c_attn (column split) + c_proj (row split) with a 4-way
ReduceScatter per batch group after c_proj; host concatenates the slices.

Per-core dataflow (all matmuls on the PE array):
  1. x[b] -> x^T via PE transposes (contraction over H needs H on partitions).
  2. QKV: Q^T,K^T [dk, S] and V [S, dk] for its 4 heads (Wq pre-scaled 1/8).
  3. Attention per head in "transposed space": S^T[k,q] = K-block @ Q^T,
     exp on ACT, causal corner mask on GPSIMD, then out^T = [V|1].T @ A_un^T.
     The appended ones-column makes the softmax denominators fall out of the
     same matmul (PSUM row 64); normalization is a DMA broadcast + one
     multiply on the tiny O^T, not a pass over the scores.
  4. Partial c_proj, ReduceScatter(add) over the 4-core batch group.

Everything after x^T is pipelined over 512-wide query chunks j: QKV(j) ->
attention(j) -> c_proj(j) -> ReduceScatter(j), so the collective overlaps
compute and the PE stream stays dense.
"""

import sys

sys.path.insert(0, "/opt/trn_rl_repo")

import numpy as np

import concourse.bass as bass
import concourse.mybir as mybir
import concourse.tile as tile
from concourse import bacc
from concourse.bass_utils import run_bass_kernel_spmd
from concourse.masks import make_identity

B, S, H, NH, DK = 2, 2048, 1024, 16, 64
NCORES = 8
HPC = 4            # heads per core
CW = HPC * DK      # 256 qkv columns per core
SLICE = S // 4     # 512 output rows per core
GROUPS = [[0, 1, 2, 3], [4, 5, 6, 7]]

F32 = mybir.dt.float32
DT_MM = mybir.dt.float32r  # matmul operand dtype: float32 | float32r


def _bcast_ap(src_ap, parts):
    """Partition-broadcast view: repeat src_ap's single row across `parts`."""
    ap = [list(p) for p in src_ap.ap]
    if len(ap) > 1 and ap[0][1] == 1:
        ap = ap[1:]  # drop singleton partition dim
    return bass.AP(
        tensor=src_ap.tensor,
        offset=src_ap.offset,
        ap=[[0, parts]] + ap,
    )


def build_nc(dt_mm=DT_MM):
    nc = bacc.Bacc(None, target_bir_lowering=False, debug=False, num_devices=NCORES)

    xb = nc.declare_dram_parameter("xb", [S, H], dt_mm, isOutput=False)
    wq = nc.declare_dram_parameter("wq", [H, CW], dt_mm, isOutput=False)
    wk = nc.declare_dram_parameter("wk", [H, CW], dt_mm, isOutput=False)
    wv = nc.declare_dram_parameter("wv", [H, CW], dt_mm, isOutput=False)
    wp = nc.declare_dram_parameter("wp", [CW, H], dt_mm, isOutput=False)
    bq = nc.declare_dram_parameter("bq", [CW], F32, isOutput=False)
    bk = nc.declare_dram_parameter("bk", [CW], F32, isOutput=False)
    bv = nc.declare_dram_parameter("bv", [CW], F32, isOutput=False)
    out = nc.declare_dram_parameter("out", [SLICE, H], F32, isOutput=True)

    KT = H // 128   # 8 contraction tiles over H
    ST = S // 128   # 16 seq tiles
    NQ = S // 512   # 4 query chunks of 512

    with tile.TileContext(nc) as tc:
        with (
            tc.tile_pool(name="dram", bufs=1, space="DRAM") as dram,
            tc.tile_pool(name="psum", bufs=1, space="PSUM") as psum,
            tc.tile_pool(name="persist", bufs=1) as pw,
        ):
            ident = pw.tile([128, 128], dt_mm)
            if dt_mm != F32:
                ident_f32 = pw.tile([128, 128], F32)
                make_identity(nc, ident_f32)
                nc.vector.tensor_copy(ident, ident_f32)
            else:
                make_identity(nc, ident)
            ones4 = pw.tile([128, HPC, 1], F32)
            nc.gpsimd.memset(ones4, 1.0)

            # weights: [128, k-tile, cols]
            wq_sb = pw.tile([128, KT, CW], dt_mm)
            wk_sb = pw.tile([128, KT, CW], dt_mm)
            wv_sb = pw.tile([128, KT, CW], dt_mm)
            wp_sb = pw.tile([128, 2, H], dt_mm)
            nc.sync.dma_start(out=wq_sb, in_=wq.ap().rearrange("(k p) c -> p k c", p=128))
            nc.sync.dma_start(out=wk_sb, in_=wk.ap().rearrange("(k p) c -> p k c", p=128))
            nc.sync.dma_start(out=wv_sb, in_=wv.ap().rearrange("(k p) c -> p k c", p=128))
            nc.sync.dma_start(out=wp_sb, in_=wp.ap().rearrange("(k p) c -> p k c", p=128))

            # biases
            bq_sb = pw.tile([128, 2], F32)
            bk_sb = pw.tile([128, 2], F32)
            nc.gpsimd.dma_start(out=bq_sb, in_=bq.ap().rearrange("(h p) -> p h", p=128))
            nc.gpsimd.dma_start(out=bk_sb, in_=bk.ap().rearrange("(h p) -> p h", p=128))
            bv_bc = pw.tile([128, CW], F32)
            nc.gpsimd.dma_start(out=bv_bc, in_=_bcast_ap(bv.ap(), 128))

            QTt = pw.tile([128, 2, S], dt_mm)   # q-col (128) x [half, s]
            KTt = pw.tile([128, 2, S], dt_mm)
            V4 = pw.tile([128, ST, HPC, DK + 1], dt_mm)  # [s-part, s-tile, head, dk|1]
            xT = pw.tile([128, KT, S], dt_mm)   # h-part x [h-tile, s]
            OTP = [pw.tile([128, S], dt_mm, name=f"otp{p}") for p in range(2)]

            # ---- phase 0: x^T ----
            for si in range(ST):
                xs = pw.tile([128, H], dt_mm, tag="xs", bufs=2)
                nc.sync.dma_start(out=xs, in_=xb[si * 128:(si + 1) * 128, :])
                for k in range(KT):
                    pt = psum.tile([128, 128], dt_mm, tag="tp", bufs=2)
                    nc.tensor.transpose(pt, xs[:, k * 128:(k + 1) * 128], ident)
                    nc.vector.tensor_copy(xT[:, k, si * 128:(si + 1) * 128], pt)

            def emit_proj(j):
                yp_j = dram.tile([512, H], F32, name=f"yp{j}")
                rs_j = dram.tile([128, H], F32, name=f"rs{j}")
                for si in range(4 * j, 4 * j + 4):
                    yt = pw.tile([128, H], F32, tag="y", bufs=2, name=f"yt{j}_{si}")
                    for n in range(2):
                        py = psum.tile([128, 512], F32, tag="qkv", bufs=2,
                                       name=f"py{j}_{si}_{n}")
                        for pair in range(2):
                            nc.tensor.matmul(
                                py,
                                OTP[pair][:, si * 128:(si + 1) * 128],
                                wp_sb[:, pair, n * 512:(n + 1) * 512],
                                start=(pair == 0), stop=(pair == 1),
                            )
                        nc.vector.tensor_copy(yt[:, n * 512:(n + 1) * 512], py)
                    nc.sync.dma_start(
                        out=yp_j[(si - 4 * j) * 128:(si - 4 * j + 1) * 128, :],
                        in_=yt)
                nc.gpsimd.collective_compute(
                    "ReduceScatter",
                    mybir.AluOpType.add,
                    replica_groups=GROUPS,
                    ins=[yp_j[:, :].opt()],
                    outs=[rs_j[:, :].opt()],
                )
                nc.gpsimd.dma_start(
                    out=out[j * 128:(j + 1) * 128, :], in_=rs_j[:, :])

            # ---- phases 1-3, pipelined over query chunks j ----
            for j in range(NQ):
                js = slice(j * 512, (j + 1) * 512)

                # QKV for chunk j
                for (wt, dst, bias) in ((wq_sb, QTt, bq_sb), (wk_sb, KTt, bk_sb)):
                    for half in range(2):
                        pq = psum.tile([128, 512], F32, tag="qkv", bufs=2)
                        for k in range(KT):
                            nc.tensor.matmul(
                                pq,
                                wt[:, k, half * 128:(half + 1) * 128],
                                xT[:, k, js],
                                start=(k == 0),
                                stop=(k == KT - 1),
                            )
                        nc.scalar.activation(
                            dst[:, half, js], pq,
                            mybir.ActivationFunctionType.Identity,
                            bias=bias[:, half:half + 1],
                        )
                for si in range(4 * j, 4 * j + 4):
                    pv = psum.tile([128, CW], F32, tag="qkv", bufs=2)
                    for k in range(KT):
                        nc.tensor.matmul(
                            pv, xT[:, k, si * 128:(si + 1) * 128], wv_sb[:, k, :],
                            start=(k == 0), stop=(k == KT - 1),
                        )
                    pv_h = pv[:, :].rearrange("p (h d) -> p h d", h=HPC)
                    bv_h = bv_bc[:, :].rearrange("p (h d) -> p h d", h=HPC)
                    nc.vector.tensor_add(V4[:, si, :, 0:DK], pv_h, bv_h)
                    nc.vector.tensor_copy(V4[:, si, :, DK:DK + 1], ones4)

                # attention chunk j, all 4 heads
                combs = []
                for h in range(HPC):
                    pb = 64 * (h % 2)
                    qt = QTt[pb:pb + DK, h // 2, :]
                    kt = KTt[pb:pb + DK, h // 2, :]
                    comb = pw.tile([65, 512], dt_mm, tag="comb", bufs=5,
                                   name=f"comb{j}_{h}")
                    combs.append(comb)
                    pav = psum.tile([65, 512], F32, tag="av", bufs=2)
                    nblk = 4 * j + 4
                    for ki in range(nblk):
                        off = max(0, 128 * ki - 512 * j)
                        npp = 512 - off
                        ps = psum.tile([128, 512], F32, tag="sT", bufs=2)
                        nc.tensor.matmul(
                            ps[:, :npp],
                            kt[:, ki * 128:(ki + 1) * 128],
                            qt[:, j * 512 + off:(j + 1) * 512],
                            start=True, stop=True,
                        )
                        A = pw.tile([128, 512], dt_mm, tag="A", bufs=3)
                        nc.scalar.activation(
                            A[:, :npp], ps[:, :npp],
                            mybir.ActivationFunctionType.Exp)
                        if ki >= 4 * j:
                            # corner: keep q_local - k_local >= 0 else 0
                            nc.gpsimd.affine_select(
                                out=A[:, :128], in_=A[:, :128],
                                compare_op=mybir.AluOpType.is_ge,
                                fill=0.0, base=0,
                                pattern=[[1, 128]], channel_multiplier=-1,
                            )
                        nc.tensor.matmul(
                            pav[:, off:],
                            V4[:, ki, h, :],
                            A[:, :npp],
                            start=(ki == 0), stop=(ki == nblk - 1),
                        )
                    nc.vector.tensor_copy(comb, pav)

                # normalization for chunk j: batch the 4 heads' reciprocals
                sums_j = dram.tile([HPC * 512], dt_mm, name=f"sums{j}")
                rsums_j = dram.tile([HPC * 512], dt_mm, name=f"rsums{j}")
                for h in range(HPC):
                    nc.sync.dma_start(
                        out=sums_j[h * 512:(h + 1) * 512], in_=combs[h][64:65, :])
                sre = pw.tile([128, HPC * 4], dt_mm, tag="sre", bufs=2)
                nc.sync.dma_start(
                    out=sre, in_=sums_j[:].rearrange("(p f) -> p f", p=128))
                with nc.allow_low_precision(reason="f32r recip of O(1e3) softmax sums"):
                    nc.vector.reciprocal(sre, sre)
                nc.sync.dma_start(
                    out=rsums_j[:].rearrange("(p f) -> p f", p=128), in_=sre)
                for h in range(HPC):
                    pair, pb = h // 2, 64 * (h % 2)
                    rbc = pw.tile([64, 512], dt_mm, tag="rbc", bufs=3)
                    nc.gpsimd.dma_start(
                        out=rbc, in_=_bcast_ap(rsums_j[h * 512:(h + 1) * 512], 64))
                    nc.vector.tensor_mul(combs[h][0:64, :], combs[h][0:64, :], rbc)
                    nc.sync.dma_start(
                        out=OTP[pair][pb:pb + 64, js], in_=combs[h][0:64, :])

                # c_proj lags one chunk so the PE never waits on the
                # normalization DMA chain of the current chunk
                if j >= 1:
                    emit_proj(j - 1)
            emit_proj(NQ - 1)

    nc.compile()
    return nc


_NC = None


def kernel(x, w_attn, b_attn, w_proj, b_proj):
    global _NC
    if _NC is None:
        _NC = build_nc()

    x = np.ascontiguousarray(np.asarray(x, dtype=np.float32))
    w_attn = np.asarray(w_attn, dtype=np.float32)
    b_attn = np.asarray(b_attn, dtype=np.float32)
    w_proj = np.asarray(w_proj, dtype=np.float32)
    b_proj = np.asarray(b_proj, dtype=np.float32)

    in_maps = []
    for c in range(NCORES):
        b, g = divmod(c, 4)
        h0 = g * HPC
        cs = slice(h0 * DK, h0 * DK + CW)
        in_maps.append({
            "xb": x[b],
            # fold the 1/sqrt(DK)=2^-3 score scale into Wq/bq (exact in fp32)
            "wq": np.ascontiguousarray(w_attn[:, cs]) * np.float32(0.125),
            "wk": np.ascontiguousarray(w_attn[:, H:][:, cs]),
            "wv": np.ascontiguousarray(w_attn[:, 2 * H:][:, cs]),
            "wp": np.ascontiguousarray(w_proj[cs.start:cs.stop, :]),
            "bq": np.ascontiguousarray(b_attn[cs]) * np.float32(0.125),
            "bk": np.ascontiguousarray(b_attn[H:][cs]),
            "bv": np.ascontiguousarray(b_attn[2 * H:][cs]),
        })

    res = run_bass_kernel_spmd(_NC, in_maps, core_ids=list(range(NCORES)))

    outp = np.empty((B, S, H), dtype=np.float32)
    for c in range(NCORES):
        b, g = divmod(c, 4)
        # out rows (j, r): global s = 512*j + 128*g + r
        arr = res.results[c]["out"].reshape(4, 128, H)
        for j in range(4):
            outp[b, 512 * j + 128 * g:512 * j + 128 * g + 128, :] = arr[j]
    outp += b_proj  # row-broadcast add, exact
    return outp


# revision 17
# speedup vs baseline: 1.9996x; 1.0690x over previous
"""Causal self-attention (B=2, S=2048, H=1024, NH=16) on 8 TRN2 NeuronCores.

Sharding: core c handles batch b = c//4 and heads [4*(c%4), 4*(c%4)+4).
Tensor-parallel c_attn (column split) + c_proj (row split) with a 4-way
ReduceScatter per batch group after c_proj; host concatenates the slices.

Per-core dataflow (all matmuls on the PE array):
  1. x[b] -> x^T via PE transposes (contraction over H needs H on partitions).
  2. QKV: Q^T,K^T [dk, S] and V [S, dk] for its 4 heads (Wq pre-scaled 1/8).
  3. Attention per head in "transposed space": S^T[k,q] = K-block @ Q^T,
     exp on ACT, causal corner mask on GPSIMD, then out^T = [V|1].T @ A_un^T.
     The appended ones-column makes the softmax denominators fall out of the
     same matmul (PSUM row 64); normalization is a DMA broadcast + one
     multiply on the tiny O^T, not a pass over the scores.
  4. Partial c_proj, ReduceScatter(add) over the 4-core batch group.

Everything after x^T is pipelined over 512-wide query chunks j: QKV(j) ->
attention(j) -> c_proj(j) -> ReduceScatter(j), so the collective overlaps
compute and the PE stream stays dense.
"""

import sys

sys.path.insert(0, "/opt/trn_rl_repo")

import numpy as np

import concourse.bass as bass
import concourse.mybir as mybir
import concourse.tile as tile
from concourse import bacc
from concourse.bass_utils import run_bass_kernel_spmd
from concourse.masks import make_identity

B, S, H, NH, DK = 2, 2048, 1024, 16, 64
NCORES = 8
HPC = 4            # heads per core
CW = HPC * DK      # 256 qkv columns per core
SLICE = S // 4     # 512 output rows per core
GROUPS = [[0, 1, 2, 3], [4, 5, 6, 7]]

F32 = mybir.dt.float32
DT_MM = mybir.dt.float32r  # matmul operand dtype: float32 | float32r


def _bcast_ap(src_ap, parts):
    """Partition-broadcast view: repeat src_ap's single row across `parts`."""
    ap = [list(p) for p in src_ap.ap]
    if len(ap) > 1 and ap[0][1] == 1:
        ap = ap[1:]  # drop singleton partition dim
    return bass.AP(
        tensor=src_ap.tensor,
        offset=src_ap.offset,
        ap=[[0, parts]] + ap,
    )


def build_nc(dt_mm=DT_MM):
    nc = bacc.Bacc(None, target_bir_lowering=False, debug=False, num_devices=NCORES)

    xb = nc.declare_dram_parameter("xb", [S, H], dt_mm, isOutput=False)
    wq = nc.declare_dram_parameter("wq", [H, CW], dt_mm, isOutput=False)
    wk = nc.declare_dram_parameter("wk", [H, CW], dt_mm, isOutput=False)
    wv = nc.declare_dram_parameter("wv", [H, CW], dt_mm, isOutput=False)
    wp = nc.declare_dram_parameter("wp", [CW, H], dt_mm, isOutput=False)
    bq = nc.declare_dram_parameter("bq", [CW], F32, isOutput=False)
    bk = nc.declare_dram_parameter("bk", [CW], F32, isOutput=False)
    bv = nc.declare_dram_parameter("bv", [CW], F32, isOutput=False)
    out = nc.declare_dram_parameter("out", [SLICE, H], F32, isOutput=True)

    KT = H // 128   # 8 contraction tiles over H
    ST = S // 128   # 16 seq tiles
    NQ = S // 512   # 4 query chunks of 512

    with tile.TileContext(nc) as tc:
        with (
            tc.tile_pool(name="dram", bufs=1, space="DRAM") as dram,
            tc.tile_pool(name="psum", bufs=1, space="PSUM") as psum,
            tc.tile_pool(name="persist", bufs=1) as pw,
        ):
            ident = pw.tile([128, 128], dt_mm)
            if dt_mm != F32:
                ident_f32 = pw.tile([128, 128], F32)
                make_identity(nc, ident_f32)
                nc.vector.tensor_copy(ident, ident_f32)
            else:
                make_identity(nc, ident)
            ones4 = pw.tile([128, HPC, 1], F32)
            nc.gpsimd.memset(ones4, 1.0)
            # lower-triangle-in-q mask: tri[k, q] = 1 if q >= k else 0
            tri_f32 = pw.tile([128, 128], F32)
            nc.gpsimd.memset(tri_f32, 1.0)
            nc.gpsimd.affine_select(
                out=tri_f32, in_=tri_f32, compare_op=mybir.AluOpType.is_ge,
                fill=0.0, base=0, pattern=[[1, 128]], channel_multiplier=-1)
            tri = pw.tile([128, 128], dt_mm)
            nc.vector.tensor_copy(tri, tri_f32)

            # weights: [128, k-tile, cols]
            wq_sb = pw.tile([128, KT, CW], dt_mm)
            wk_sb = pw.tile([128, KT, CW], dt_mm)
            wv_sb = pw.tile([128, KT, CW], dt_mm)
            wp_sb = pw.tile([128, 2, H], dt_mm)
            nc.gpsimd.dma_start(out=wq_sb, in_=wq.ap().rearrange("(k p) c -> p k c", p=128))
            nc.gpsimd.dma_start(out=wk_sb, in_=wk.ap().rearrange("(k p) c -> p k c", p=128))
            nc.gpsimd.dma_start(out=wv_sb, in_=wv.ap().rearrange("(k p) c -> p k c", p=128))
            nc.gpsimd.dma_start(out=wp_sb, in_=wp.ap().rearrange("(k p) c -> p k c", p=128))

            # biases
            bq_sb = pw.tile([128, 2], F32)
            bk_sb = pw.tile([128, 2], F32)
            nc.gpsimd.dma_start(out=bq_sb, in_=bq.ap().rearrange("(h p) -> p h", p=128))
            nc.gpsimd.dma_start(out=bk_sb, in_=bk.ap().rearrange("(h p) -> p h", p=128))
            bv_bc = pw.tile([128, CW], F32)
            nc.gpsimd.dma_start(out=bv_bc, in_=_bcast_ap(bv.ap(), 128))

            QTt = pw.tile([128, 2, S], dt_mm)   # q-col (128) x [half, s]
            KTt = pw.tile([128, 2, S], dt_mm)
            V4 = pw.tile([128, ST, HPC, DK + 1], dt_mm)  # [s-part, s-tile, head, dk|1]
            xT = pw.tile([128, KT, S], dt_mm)   # h-part x [h-tile, s]
            OTP = [pw.tile([128, S], dt_mm, name=f"otp{p}") for p in range(2)]

            # ---- phase 0: x^T ----
            for si in range(ST):
                xs = pw.tile([128, H], dt_mm, tag="xs", bufs=2)
                nc.sync.dma_start(out=xs, in_=xb[si * 128:(si + 1) * 128, :])
                for k in range(KT):
                    pt = psum.tile([128, 128], dt_mm, tag="tpav", bufs=4)
                    nc.tensor.transpose(pt, xs[:, k * 128:(k + 1) * 128], ident)
                    nc.vector.tensor_copy(xT[:, k, si * 128:(si + 1) * 128], pt)

            def emit_proj(j):
                yp_j = dram.tile([512, H], F32, name=f"yp{j}")
                rs_j = dram.tile([128, H], F32, name=f"rs{j}")
                for si in range(4 * j, 4 * j + 4):
                    yt = pw.tile([128, H], F32, tag="y", bufs=2, name=f"yt{j}_{si}")
                    for n in range(2):
                        py = psum.tile([128, 512], F32, tag="qkv", bufs=2,
                                       name=f"py{j}_{si}_{n}")
                        for pair in range(2):
                            nc.tensor.matmul(
                                py,
                                OTP[pair][:, si * 128:(si + 1) * 128],
                                wp_sb[:, pair, n * 512:(n + 1) * 512],
                                start=(pair == 0), stop=(pair == 1),
                            )
                        nc.vector.tensor_copy(yt[:, n * 512:(n + 1) * 512], py)
                    nc.sync.dma_start(
                        out=yp_j[(si - 4 * j) * 128:(si - 4 * j + 1) * 128, :],
                        in_=yt)
                nc.gpsimd.collective_compute(
                    "ReduceScatter",
                    mybir.AluOpType.add,
                    replica_groups=GROUPS,
                    ins=[yp_j[:, :].opt()],
                    outs=[rs_j[:, :].opt()],
                )
                rs_tiles.append((j, rs_j))

            # ---- phases 1-3, pipelined over query chunks j ----
            rs_tiles = []
            for j in range(NQ):
                js = slice(j * 512, (j + 1) * 512)

                # QKV for chunk j
                for (wt, dst, bias) in ((wq_sb, QTt, bq_sb), (wk_sb, KTt, bk_sb)):
                    for half in range(2):
                        pq = psum.tile([128, 512], F32, tag="qkv", bufs=2)
                        for k in range(KT):
                            nc.tensor.matmul(
                                pq,
                                wt[:, k, half * 128:(half + 1) * 128],
                                xT[:, k, js],
                                start=(k == 0),
                                stop=(k == KT - 1),
                            )
                        nc.scalar.activation(
                            dst[:, half, js], pq,
                            mybir.ActivationFunctionType.Identity,
                            bias=bias[:, half:half + 1],
                        )
                for si in range(4 * j, 4 * j + 4):
                    pv = psum.tile([128, CW], F32, tag="qkv", bufs=2)
                    for k in range(KT):
                        nc.tensor.matmul(
                            pv, xT[:, k, si * 128:(si + 1) * 128], wv_sb[:, k, :],
                            start=(k == 0), stop=(k == KT - 1),
                        )
                    pv_h = pv[:, :].rearrange("p (h d) -> p h d", h=HPC)
                    bv_h = bv_bc[:, :].rearrange("p (h d) -> p h d", h=HPC)
                    nc.vector.tensor_add(V4[:, si, :, 0:DK], pv_h, bv_h)
                    nc.vector.tensor_copy(V4[:, si, :, DK:DK + 1], ones4)

                # attention chunk j, all 4 heads
                combs = []
                for h in range(HPC):
                    pb = 64 * (h % 2)
                    qt = QTt[pb:pb + DK, h // 2, :]
                    kt = KTt[pb:pb + DK, h // 2, :]
                    comb = pw.tile([65, 512], dt_mm, tag="comb", bufs=5,
                                   name=f"comb{j}_{h}")
                    combs.append(comb)
                    pav = psum.tile([65, 512], F32, tag="tpav", bufs=4)
                    nblk = 4 * j + 4
                    for ki in range(nblk):
                        off = max(0, 128 * ki - 512 * j)
                        npp = 512 - off
                        ps = psum.tile([128, 512], F32, tag="sT", bufs=2)
                        nc.tensor.matmul(
                            ps[:, :npp],
                            kt[:, ki * 128:(ki + 1) * 128],
                            qt[:, j * 512 + off:(j + 1) * 512],
                            start=True, stop=True,
                        )
                        A = pw.tile([128, 512], dt_mm, tag="A", bufs=4)
                        nc.scalar.activation(
                            A[:, :npp], ps[:, :npp],
                            mybir.ActivationFunctionType.Exp)
                        if ki >= 4 * j:
                            # corner: zero the k > q triangle
                            nc.vector.tensor_mul(A[:, :128], A[:, :128], tri)
                        nc.tensor.matmul(
                            pav[:, off:],
                            V4[:, ki, h, :],
                            A[:, :npp],
                            start=(ki == 0), stop=(ki == nblk - 1),
                        )
                    nc.vector.tensor_copy(comb, pav)

                # normalization for chunk j: batch the 4 heads' reciprocals
                sums_j = dram.tile([HPC * 512], dt_mm, name=f"sums{j}")
                rsums_j = dram.tile([HPC * 512], dt_mm, name=f"rsums{j}")
                for h in range(HPC):
                    nc.sync.dma_start(
                        out=sums_j[h * 512:(h + 1) * 512], in_=combs[h][64:65, :])
                sre = pw.tile([128, HPC * 4], dt_mm, tag="sre", bufs=2)
                nc.sync.dma_start(
                    out=sre, in_=sums_j[:].rearrange("(p f) -> p f", p=128))
                with nc.allow_low_precision(reason="f32r recip of O(1e3) softmax sums"):
                    nc.vector.reciprocal(sre, sre)
                nc.sync.dma_start(
                    out=rsums_j[:].rearrange("(p f) -> p f", p=128), in_=sre)
                for h in range(HPC):
                    pair, pb = h // 2, 64 * (h % 2)
                    rbc = pw.tile([64, 512], dt_mm, tag="rbc", bufs=3)
                    nc.gpsimd.dma_start(
                        out=rbc, in_=_bcast_ap(rsums_j[h * 512:(h + 1) * 512], 64))
                    nc.vector.tensor_mul(combs[h][0:64, :], combs[h][0:64, :], rbc)
                    nc.sync.dma_start(
                        out=OTP[pair][pb:pb + 64, js], in_=combs[h][0:64, :])

                # c_proj lags one chunk so the PE never waits on the
                # normalization DMA chain of the current chunk
                if j >= 1:
                    emit_proj(j - 1)
            emit_proj(NQ - 1)
            for j, rs_j in rs_tiles:
                nc.gpsimd.dma_start(
                    out=out[j * 128:(j + 1) * 128, :], in_=rs_j[:, :])

    nc.compile()
    return nc


_NC = None


def kernel(x, w_attn, b_attn, w_proj, b_proj):
    global _NC
    if _NC is None:
        _NC = build_nc()

    x = np.ascontiguousarray(np.asarray(x, dtype=np.float32))
    w_attn = np.asarray(w_attn, dtype=np.float32)
    b_attn = np.asarray(b_attn, dtype=np.float32)
    w_proj = np.asarray(w_proj, dtype=np.float32)
    b_proj = np.asarray(b_proj, dtype=np.float32)

    in_maps = []
    for c in range(NCORES):
        b, g = divmod(c, 4)
        h0 = g * HPC
        cs = slice(h0 * DK, h0 * DK + CW)
        in_maps.append({
            "xb": x[b],
            # fold the 1/sqrt(DK)=2^-3 score scale into Wq/bq (exact in fp32)
            "wq": np.ascontiguousarray(w_attn[:, cs]) * np.float32(0.125),
            "wk": np.ascontiguousarray(w_attn[:, H:][:, cs]),
            "wv": np.ascontiguousarray(w_attn[:, 2 * H:][:, cs]),
            "wp": np.ascontiguousarray(w_proj[cs.start:cs.stop, :]),
            "bq": np.ascontiguousarray(b_attn[cs]) * np.float32(0.125),
            "bk": np.ascontiguousarray(b_attn[H:][cs]),
            "bv": np.ascontiguousarray(b_attn[2 * H:][cs]),
        })

    res = run_bass_kernel_spmd(_NC, in_maps, core_ids=list(range(NCORES)))

    outp = np.empty((B, S, H), dtype=np.float32)
    for c in range(NCORES):
        b, g = divmod(c, 4)
        # out rows (j, r): global s = 512*j + 128*g + r
        arr = res.results[c]["out"].reshape(4, 128, H)
        for j in range(4):
            outp[b, 512 * j + 128 * g:512 * j + 128 * g + 128, :] = arr[j]
    outp += b_proj  # row-broadcast add, exact
    return outp


# revision 18
# speedup vs baseline: 2.0200x; 1.0102x over previous
"""Causal self-attention (B=2, S=2048, H=1024, NH=16) on 8 TRN2 NeuronCores.

Sharding: core c handles batch b = c//4 and heads [4*(c%4), 4*(c%4)+4).
Tensor-parallel c_attn (column split) + c_proj (row split) with a 4-way
ReduceScatter per batch group after c_proj; host concatenates the slices.

Per-core dataflow (all matmuls on the PE array):
  1. x[b] -> x^T via PE transposes (contraction over H needs H on partitions).
  2. QKV: Q^T,K^T [dk, S] and V [S, dk] for its 4 heads (Wq pre-scaled 1/8).
  3. Attention per head in "transposed space": S^T[k,q] = K-block @ Q^T,
     exp on ACT, causal corner mask on GPSIMD, then out^T = [V|1].T @ A_un^T.
     The appended ones-column makes the softmax denominators fall out of the
     same matmul (PSUM row 64); normalization is a DMA broadcast + one
     multiply on the tiny O^T, not a pass over the scores.
  4. Partial c_proj, ReduceScatter(add) over the 4-core batch group.

Everything after x^T is pipelined over 512-wide query chunks j: QKV(j) ->
attention(j) -> c_proj(j) -> ReduceScatter(j), so the collective overlaps
compute and the PE stream stays dense.
"""

import sys

sys.path.insert(0, "/opt/trn_rl_repo")

import numpy as np

import concourse.bass as bass
import concourse.mybir as mybir
import concourse.tile as tile
from concourse import bacc
from concourse.bass_utils import run_bass_kernel_spmd
from concourse.masks import make_identity

B, S, H, NH, DK = 2, 2048, 1024, 16, 64
NCORES = 8
HPC = 4            # heads per core
CW = HPC * DK      # 256 qkv columns per core
SLICE = S // 4     # 512 output rows per core
GROUPS = [[0, 1, 2, 3], [4, 5, 6, 7]]

F32 = mybir.dt.float32
DT_MM = mybir.dt.float32r  # matmul operand dtype: float32 | float32r


def _bcast_ap(src_ap, parts):
    """Partition-broadcast view: repeat src_ap's single row across `parts`."""
    ap = [list(p) for p in src_ap.ap]
    if len(ap) > 1 and ap[0][1] == 1:
        ap = ap[1:]  # drop singleton partition dim
    return bass.AP(
        tensor=src_ap.tensor,
        offset=src_ap.offset,
        ap=[[0, parts]] + ap,
    )


def build_nc(dt_mm=DT_MM):
    nc = bacc.Bacc(None, target_bir_lowering=False, debug=False, num_devices=NCORES)

    xb = nc.declare_dram_parameter("xb", [S, H], dt_mm, isOutput=False)
    wq = nc.declare_dram_parameter("wq", [H, CW], dt_mm, isOutput=False)
    wk = nc.declare_dram_parameter("wk", [H, CW], dt_mm, isOutput=False)
    wv = nc.declare_dram_parameter("wv", [H, CW], dt_mm, isOutput=False)
    wp = nc.declare_dram_parameter("wp", [CW, H], dt_mm, isOutput=False)
    bq = nc.declare_dram_parameter("bq", [CW], F32, isOutput=False)
    bk = nc.declare_dram_parameter("bk", [CW], F32, isOutput=False)
    bv = nc.declare_dram_parameter("bv", [CW], F32, isOutput=False)
    out = nc.declare_dram_parameter("out", [SLICE, H], F32, isOutput=True)

    KT = H // 128   # 8 contraction tiles over H
    ST = S // 128   # 16 seq tiles
    NQ = S // 512   # 4 query chunks of 512

    with tile.TileContext(nc) as tc:
        with (
            tc.tile_pool(name="dram", bufs=1, space="DRAM") as dram,
            tc.tile_pool(name="psum", bufs=1, space="PSUM") as psum,
            tc.tile_pool(name="persist", bufs=1) as pw,
        ):
            ident = pw.tile([128, 128], dt_mm)
            if dt_mm != F32:
                ident_f32 = pw.tile([128, 128], F32)
                make_identity(nc, ident_f32)
                nc.vector.tensor_copy(ident, ident_f32)
            else:
                make_identity(nc, ident)
            ones4 = pw.tile([128, HPC, 1], F32)
            nc.gpsimd.memset(ones4, 1.0)
            # lower-triangle-in-q mask: tri[k, q] = 1 if q >= k else 0
            tri_f32 = pw.tile([128, 128], F32)
            nc.gpsimd.memset(tri_f32, 1.0)
            nc.gpsimd.affine_select(
                out=tri_f32, in_=tri_f32, compare_op=mybir.AluOpType.is_ge,
                fill=0.0, base=0, pattern=[[1, 128]], channel_multiplier=-1)
            tri = pw.tile([128, 128], dt_mm)
            nc.vector.tensor_copy(tri, tri_f32)

            # weights: [128, k-tile, cols]
            wq_sb = pw.tile([128, KT, CW], dt_mm)
            wk_sb = pw.tile([128, KT, CW], dt_mm)
            wv_sb = pw.tile([128, KT, CW], dt_mm)
            wp_sb = pw.tile([128, 2, H], dt_mm)
            nc.gpsimd.dma_start(out=wq_sb, in_=wq.ap().rearrange("(k p) c -> p k c", p=128))
            nc.gpsimd.dma_start(out=wk_sb, in_=wk.ap().rearrange("(k p) c -> p k c", p=128))
            nc.gpsimd.dma_start(out=wv_sb, in_=wv.ap().rearrange("(k p) c -> p k c", p=128))
            nc.gpsimd.dma_start(out=wp_sb, in_=wp.ap().rearrange("(k p) c -> p k c", p=128))

            # biases
            bq_sb = pw.tile([128, 2], F32)
            bk_sb = pw.tile([128, 2], F32)
            nc.gpsimd.dma_start(out=bq_sb, in_=bq.ap().rearrange("(h p) -> p h", p=128))
            nc.gpsimd.dma_start(out=bk_sb, in_=bk.ap().rearrange("(h p) -> p h", p=128))
            bv_bc = pw.tile([128, CW], F32)
            nc.gpsimd.dma_start(out=bv_bc, in_=_bcast_ap(bv.ap(), 128))

            QTt = pw.tile([128, 2, S], dt_mm)   # q-col (128) x [half, s]
            KTt = pw.tile([128, 2, S], dt_mm)
            V4 = pw.tile([128, ST, HPC, DK + 1], dt_mm)  # [s-part, s-tile, head, dk|1]
            xT = pw.tile([128, KT, S], dt_mm)   # h-part x [h-tile, s]
            OTP = [pw.tile([128, S], dt_mm, name=f"otp{p}") for p in range(2)]

            # ---- phase 0: x^T ----
            for si in range(ST):
                xs = pw.tile([128, H], dt_mm, tag="xs", bufs=2)
                nc.sync.dma_start(out=xs, in_=xb[si * 128:(si + 1) * 128, :])
                for k in range(KT):
                    pt = psum.tile([128, 128], dt_mm, tag="tpav", bufs=4)
                    nc.tensor.transpose(pt, xs[:, k * 128:(k + 1) * 128], ident)
                    nc.vector.tensor_copy(xT[:, k, si * 128:(si + 1) * 128], pt)

            def emit_proj(j):
                yp_j = dram.tile([512, H], F32, name=f"yp{j}")
                rs_j = dram.tile([128, H], F32, name=f"rs{j}")
                for si in range(4 * j, 4 * j + 4):
                    yt = pw.tile([128, H], F32, tag="y", bufs=2, name=f"yt{j}_{si}")
                    for n in range(2):
                        py = psum.tile([128, 512], F32, tag="qkv", bufs=2,
                                       name=f"py{j}_{si}_{n}")
                        for pair in range(2):
                            nc.tensor.matmul(
                                py,
                                OTP[pair][:, si * 128:(si + 1) * 128],
                                wp_sb[:, pair, n * 512:(n + 1) * 512],
                                start=(pair == 0), stop=(pair == 1),
                            )
                        nc.vector.tensor_copy(yt[:, n * 512:(n + 1) * 512], py)
                    nc.sync.dma_start(
                        out=yp_j[(si - 4 * j) * 128:(si - 4 * j + 1) * 128, :],
                        in_=yt)
                nc.gpsimd.collective_compute(
                    "ReduceScatter",
                    mybir.AluOpType.add,
                    replica_groups=GROUPS,
                    ins=[yp_j[:, :].opt()],
                    outs=[rs_j[:, :].opt()],
                )
                rs_tiles.append((j, rs_j))

            # ---- phases 1-3, pipelined over query chunks j ----
            rs_tiles = []
            for j in range(NQ):
                js = slice(j * 512, (j + 1) * 512)

                # QKV for chunk j
                for (wt, dst, bias) in ((wq_sb, QTt, bq_sb), (wk_sb, KTt, bk_sb)):
                    for half in range(2):
                        pq = psum.tile([128, 512], F32, tag="qkv", bufs=2)
                        for k in range(KT):
                            nc.tensor.matmul(
                                pq,
                                wt[:, k, half * 128:(half + 1) * 128],
                                xT[:, k, js],
                                start=(k == 0),
                                stop=(k == KT - 1),
                            )
                        nc.scalar.activation(
                            dst[:, half, js], pq,
                            mybir.ActivationFunctionType.Identity,
                            bias=bias[:, half:half + 1],
                        )
                for si in range(4 * j, 4 * j + 4):
                    pv = psum.tile([128, CW], F32, tag="qkv", bufs=2)
                    for k in range(KT):
                        nc.tensor.matmul(
                            pv, xT[:, k, si * 128:(si + 1) * 128], wv_sb[:, k, :],
                            start=(k == 0), stop=(k == KT - 1),
                        )
                    pv_h = pv[:, :].rearrange("p (h d) -> p h d", h=HPC)
                    bv_h = bv_bc[:, :].rearrange("p (h d) -> p h d", h=HPC)
                    nc.vector.tensor_add(V4[:, si, :, 0:DK], pv_h, bv_h)
                    nc.vector.tensor_copy(V4[:, si, :, DK:DK + 1], ones4)

                # c_proj lags half a chunk: emitted after QKV(j) so the
                # normalize chain of chunk j-1 hides behind the QKV matmuls
                if j >= 1:
                    emit_proj(j - 1)

                # attention chunk j, all 4 heads
                combs = []
                for h in range(HPC):
                    pb = 64 * (h % 2)
                    qt = QTt[pb:pb + DK, h // 2, :]
                    kt = KTt[pb:pb + DK, h // 2, :]
                    comb = pw.tile([65, 512], dt_mm, tag="comb", bufs=5,
                                   name=f"comb{j}_{h}")
                    combs.append(comb)
                    pav = psum.tile([65, 512], F32, tag="tpav", bufs=4)
                    nblk = 4 * j + 4
                    for ki in range(nblk):
                        off = max(0, 128 * ki - 512 * j)
                        npp = 512 - off
                        ps = psum.tile([128, 512], F32, tag="sT", bufs=2)
                        nc.tensor.matmul(
                            ps[:, :npp],
                            kt[:, ki * 128:(ki + 1) * 128],
                            qt[:, j * 512 + off:(j + 1) * 512],
                            start=True, stop=True,
                        )
                        A = pw.tile([128, 512], dt_mm, tag="A", bufs=4)
                        nc.scalar.activation(
                            A[:, :npp], ps[:, :npp],
                            mybir.ActivationFunctionType.Exp)
                        if ki >= 4 * j:
                            # corner: zero the k > q triangle
                            nc.vector.tensor_mul(A[:, :128], A[:, :128], tri)
                        nc.tensor.matmul(
                            pav[:, off:],
                            V4[:, ki, h, :],
                            A[:, :npp],
                            start=(ki == 0), stop=(ki == nblk - 1),
                        )
                    nc.vector.tensor_copy(comb, pav)

                    # per-head normalization, pipelined across heads:
                    # sums -> [128,4] reciprocal -> partition-broadcast -> scale
                    pair = h // 2
                    sums_h = dram.tile([512], dt_mm, name=f"sums{j}_{h}")
                    rsums_h = dram.tile([512], dt_mm, name=f"rsums{j}_{h}")
                    nc.sync.dma_start(out=sums_h[:], in_=comb[64:65, :])
                    sre = pw.tile([128, 4], dt_mm, tag="sre", bufs=4,
                                  name=f"sre{j}_{h}")
                    nc.sync.dma_start(
                        out=sre, in_=sums_h[:].rearrange("(p f) -> p f", p=128))
                    with nc.allow_low_precision(
                            reason="f32r recip of O(1e3) softmax sums"):
                        nc.vector.reciprocal(sre, sre)
                    nc.sync.dma_start(
                        out=rsums_h[:].rearrange("(p f) -> p f", p=128), in_=sre)
                    rbc = pw.tile([64, 512], dt_mm, tag="rbc", bufs=3,
                                  name=f"rbc{j}_{h}")
                    nc.gpsimd.dma_start(out=rbc, in_=_bcast_ap(rsums_h[:], 64))
                    nc.vector.tensor_mul(comb[0:64, :], comb[0:64, :], rbc)
                    nc.sync.dma_start(
                        out=OTP[pair][pb:pb + 64, js], in_=comb[0:64, :])

            emit_proj(NQ - 1)
            for j, rs_j in rs_tiles:
                nc.gpsimd.dma_start(
                    out=out[j * 128:(j + 1) * 128, :], in_=rs_j[:, :])

    nc.compile()
    return nc


_NC = None


def kernel(x, w_attn, b_attn, w_proj, b_proj):
    global _NC
    if _NC is None:
        _NC = build_nc()

    x = np.ascontiguousarray(np.asarray(x, dtype=np.float32))
    w_attn = np.asarray(w_attn, dtype=np.float32)
    b_attn = np.asarray(b_attn, dtype=np.float32)
    w_proj = np.asarray(w_proj, dtype=np.float32)
    b_proj = np.asarray(b_proj, dtype=np.float32)

    in_maps = []
    for c in range(NCORES):
        b, g = divmod(c, 4)
        h0 = g * HPC
        cs = slice(h0 * DK, h0 * DK + CW)
        in_maps.append({
            "xb": x[b],
            # fold the 1/sqrt(DK)=2^-3 score scale into Wq/bq (exact in fp32)
            "wq": np.ascontiguousarray(w_attn[:, cs]) * np.float32(0.125),
            "wk": np.ascontiguousarray(w_attn[:, H:][:, cs]),
            "wv": np.ascontiguousarray(w_attn[:, 2 * H:][:, cs]),
            "wp": np.ascontiguousarray(w_proj[cs.start:cs.stop, :]),
            "bq": np.ascontiguousarray(b_attn[cs]) * np.float32(0.125),
            "bk": np.ascontiguousarray(b_attn[H:][cs]),
            "bv": np.ascontiguousarray(b_attn[2 * H:][cs]),
        })

    res = run_bass_kernel_spmd(_NC, in_maps, core_ids=list(range(NCORES)))

    outp = np.empty((B, S, H), dtype=np.float32)
    for c in range(NCORES):
        b, g = divmod(c, 4)
        # out rows (j, r): global s = 512*j + 128*g + r
        arr = res.results[c]["out"].reshape(4, 128, H)
        for j in range(4):
            outp[b, 512 * j + 128 * g:512 * j + 128 * g + 128, :] = arr[j]
    outp += b_proj  # row-broadcast add, exact
    return outp
